# revision 5
# baseline (speedup 1.0000x reference)
"""MoE Trainium2 kernel v5: two-segment SPMD load balancing, all-bf16.

Every core runs the SAME program shape: two token segments of sizes (a, b),
each segment paired with its own W1/W2 weight inputs. Heavy experts (whose
routed count exceeds the segment budget) are split across two cores'
a-segments; light experts pair up in b-segments. The (a, b) sizes are chosen
per-input by a tiny solver (k experts split a+a, 8-2k run a+b, k pair b+b)
minimizing a+b — for balanced routing this lands ~4% above the perfect
sum/8 split vs ~11% for one-expert-per-core.

Device math per core (both segments, bf16):
    hT = relu(W1_s.T @ xeT_s)   (F, a|b)  in one SBUF tile [128, NF, a+b]
    yT = W2_s.T @ hT_s          (D, a|b)  W2 128x128 chunks stationary

v5 over v4 (baseline 253.6us):
  - no PE warmup matmuls: they sat AHEAD of real work in the in-order PE
    queue and delayed the first real matmul ~2.5us (its DMA deps were
    ready earlier); the DVFS clock ramp happens on wall-clock regardless.
  - startup DMA issue order reworked: each dma_start costs ~600ns of
    serial sync-queue issue time, and fc=1's weights used to arrive
    ~2.4us late (one 2.4us PE gap). w1 now streams as 2-fc packs
    (halves issue count), xe as 2-k packs after the first two singles,
    and fc0/fc1 weights are interleaved with the xe chunks.
  - per-chunk PSUM pools with bufs 3/3/2 (8 banks exactly; the warmup
    pool's bank is freed) to kill periodic ~55ns bank-recycle stalls.
  - last-dh copy engines rebalanced: scalar (faster per col) takes all
    but the widest chunk, vector takes the widest; previously scalar was
    the tail critical path while vector sat idle.
"""

import math
import sys

import numpy as np

for _p in ("/opt/trn_rl_repo",):
    if _p not in sys.path:
        sys.path.append(_p)

T, E, D, F, C, K = 4096, 8, 1024, 4096, 1536, 2
N_CORES = 8
P = 128
KO = D // P
NF = F // P
NDC = D // P

_PROGRAMS = {}


def _plan_segments(n_e):
    """Pick segment sizes (a, b) and assign experts to the 8 a-segs + 8 b-segs.

    Returns (a, b, a_segs, b_segs) where each seg list has 8 entries
    (expert, src_off, cnt): the segment holds slots [src_off, src_off+cnt)
    of that expert's gain-sorted slot list (cnt may be 0 for pad segments).
    """
    ns = sorted(range(E), key=lambda e: -n_e[e])  # experts by load desc
    best = None
    for k in range(0, E // 2 + 1):
        top = [n_e[e] for e in ns[:k]]
        mid = [n_e[e] for e in ns[k : E - k]]
        bot = [n_e[e] for e in ns[E - k :]]
        a_min = max([(v + 1) // 2 for v in top], default=0)
        b_min = max([(v + 1) // 2 for v in bot], default=0)
        mid_max = max(mid, default=0)
        a = max(a_min, (mid_max + 1) // 2, 16)
        b = max(b_min, mid_max - a, 16)
        a = (a + 7) // 8 * 8
        b = (b + 7) // 8 * 8
        if best is None or a + b < best[0] + best[1]:
            best = (a, b, k)
    a, b, k = best
    a_segs, b_segs = [], []
    for i, e in enumerate(ns):
        n = n_e[e]
        if i < k:  # a + a
            a_segs.append((e, 0, min(a, n)))
            a_segs.append((e, min(a, n), max(0, n - a)))
        elif i < E - k:  # a + b
            a_segs.append((e, 0, min(a, n)))
            b_segs.append((e, min(a, n), max(0, n - a)))
        else:  # b + b
            b_segs.append((e, 0, min(b, n)))
            b_segs.append((e, min(b, n), max(0, n - b)))
    assert len(a_segs) == N_CORES and len(b_segs) == N_CORES
    return a, b, a_segs, b_segs


def _seg_chunks(width, base):
    """<=512-wide chunk list for one segment, smallest chunk first."""
    out = []
    rem, c0 = width, base
    while rem > 0:
        take = min(512, rem)
        out.append((c0, take))
        c0 += take
        rem -= take
    out.sort(key=lambda t: t[1])
    return out


def _build_program(a, b):
    import concourse.mybir as mybir
    import concourse.tile as tile
    from concourse import bacc

    f32 = mybir.dt.float32
    bf16 = mybir.dt.bfloat16
    Relu = mybir.ActivationFunctionType.Relu
    Ident = mybir.ActivationFunctionType.Identity

    B = a + b
    # per-segment chunk lists; global tag numbering across both
    seg_chunks = [_seg_chunks(a, 0), _seg_chunks(b, a)]
    flat = [(s, c0, cw) for s in (0, 1) for (c0, cw) in seg_chunks[s]]
    nflat = len(flat)

    # PSUM banks: 8 x 2KB; each <=512-wide fp32 accumulation tile is one
    # bank. Give every chunk 2 bufs, then hand the leftover banks to the
    # widest chunks (deeper rotation = fewer bank-recycle stalls).
    bufs = [2] * nflat
    spare = 8 - 2 * nflat
    for i in sorted(range(nflat), key=lambda i: -flat[i][2]):
        if spare <= 0:
            break
        bufs[i] += 1
        spare -= 1

    NT = NF // 2  # two fc per w1 pack

    nc = bacc.Bacc(None, target_bir_lowering=False, debug=False)

    with tile.TileContext(nc) as tc:
        with tc.tile_pool(name="dram", bufs=1, space="DRAM") as dram:
            # w1 packed 2 fc per tile: (NT, P, 2, KO, P)
            w1s = [
                dram.tile((NT, P, 2, KO, P), bf16, kind="ExternalInput", name=f"w1{s}")
                for s in range(2)
            ]
            # w2 packed per d-chunk: (NDC, P, NF, 128)
            w2s = [
                dram.tile((NDC, P, NF, P), bf16, kind="ExternalInput", name=f"w2{s}")
                for s in range(2)
            ]
            xeT = dram.tile((D, B), bf16, kind="ExternalInput", name="xeT")
            yT = dram.tile((D, B), bf16, kind="ExternalOutput", name="yT")

        xeT_r = xeT[:].rearrange("(ko ki) c -> ki ko c", ki=P)

        pools = [
            tc.alloc_tile_pool(name=f"ps{i}", bufs=bufs[i], space="PSUM")
            for i in range(nflat)
        ]
        with (
            tc.tile_pool(name="const", bufs=1) as constp,
            tc.tile_pool(name="xe", bufs=1) as xep,
            tc.tile_pool(name="ht", bufs=1) as htp,
            tc.tile_pool(name="ysb", bufs=1) as yp,
            tc.tile_pool(name="w1f", bufs=1) as w1fp,
            tc.tile_pool(name="w1t", bufs=4) as w1p,
            tc.tile_pool(name="w2t", bufs=4) as w2p,
        ):
            zero = constp.tile([P, 1], f32)
            nc.any.memset(zero[:], 0.0)

            xe_sb = xep.tile([P, KO, B], bf16)
            w1_first = [
                [w1fp.tile([P, KO, P], bf16, name=f"w1f{fc}{s}") for s in range(2)]
                for fc in range(2)
            ]
            w1_packs = {}

            def issue_pack(t):
                w1_packs[t] = [
                    w1p.tile([P, 2, KO, P], bf16, name="w1pk", tag=f"w1pk{s}")
                    for s in range(2)
                ]
                for s in range(2):
                    nc.sync.dma_start(w1_packs[t][s][:], w1s[s][t])

            # Startup issue order: each dma_start costs ~600ns of serial
            # sync-queue time, so interleave first-needed xe chunks with
            # the fc0/fc1 weights, then packed (2-k) xe chunks, then the
            # first three w1 packs.
            nc.sync.dma_start(xe_sb[:, 0, :], xeT_r[:, 0, :])
            for s in range(2):
                nc.sync.dma_start(w1_first[0][s][:], w1s[s][0][:, 0])
            nc.sync.dma_start(xe_sb[:, 1, :], xeT_r[:, 1, :])
            for s in range(2):
                nc.sync.dma_start(w1_first[1][s][:], w1s[s][0][:, 1])
            for j in range(1, KO // 2):
                nc.sync.dma_start(
                    xe_sb[:, 2 * j : 2 * j + 2, :], xeT_r[:, 2 * j : 2 * j + 2, :]
                )
            for t in range(1, 4):
                issue_pack(t)

            hT = htp.tile([P, NF, B], bf16)
            yT_sb = yp.tile([P, NDC, B], bf16)

            # W2 d-chunk tiles are streamed; first two d-chunks prefetch
            # during the tail of MM1 (after the w1 tiles they'd contend with)
            w2_t = {}

            # ---- MM1 ----
            for fc in range(NF):
                t, sub = divmod(fc, 2)
                if sub == 0 and t >= 1 and t + 3 < NT:
                    issue_pack(t + 3)
                if fc < 2:
                    w1_t = [w1_first[fc][s][:] for s in range(2)]
                else:
                    w1_t = [w1_packs[t][s][:, sub] for s in range(2)]
                if fc >= NF - 4:
                    dh, s = divmod(fc - (NF - 4), 2)
                    w2_t[(dh, s)] = w2p.tile([P, NF, P], bf16, name="w2_t")
                    nc.sync.dma_start(w2_t[(dh, s)][:], w2s[s][dh])
                ph = {
                    i: pools[i].tile([P, cw], f32, name=f"p{i}", tag=f"p{i}")
                    for i, (s, c0, cw) in enumerate(flat)
                }
                for k in range(KO):
                    for i, (s, c0, cw) in enumerate(flat):
                        nc.tensor.matmul(
                            ph[i][:],
                            w1_t[s][:, k, :],
                            xe_sb[:, k, c0 : c0 + cw],
                            start=(k == 0),
                            stop=(k == KO - 1),
                        )
                for i, (s, c0, cw) in enumerate(flat):
                    nc.scalar.activation(
                        hT[:, fc, c0 : c0 + cw], ph[i][:], Relu, bias=zero[:]
                    )

            # ---- MM2 ----
            widest = max(range(nflat), key=lambda i: flat[i][2])
            for dh in range(NDC):
                if dh + 2 < NDC:
                    for s in range(2):
                        w2_t[(dh + 2, s)] = w2p.tile([P, NF, P], bf16, name="w2_t")
                        nc.sync.dma_start(w2_t[(dh + 2, s)][:], w2s[s][dh + 2])
                py = {
                    i: pools[i].tile([P, cw], f32, name=f"py{i}", tag=f"p{i}")
                    for i, (s, c0, cw) in enumerate(flat)
                }
                for fs in range(NF):
                    for i, (s, c0, cw) in enumerate(flat):
                        nc.tensor.matmul(
                            py[i][:],
                            w2_t[(dh, s)][:, fs, :],
                            hT[:, fs, c0 : c0 + cw],
                            start=(fs == 0),
                            stop=(fs == NF - 1),
                        )
                for i, (s, c0, cw) in enumerate(flat):
                    dst = yT_sb[:, dh, c0 : c0 + cw]
                    if dh == NDC - 1:
                        # tail critical path: scalar is faster per column,
                        # so it takes everything except the widest chunk;
                        # vector (otherwise idle) takes the widest.
                        if i == widest:
                            nc.vector.tensor_copy(dst, py[i][:])
                        else:
                            nc.scalar.activation(dst, py[i][:], Ident, bias=zero[:])
                    elif i % 2 == 0:
                        nc.vector.tensor_copy(dst, py[i][:])
                    else:
                        nc.scalar.activation(dst, py[i][:], Ident, bias=zero[:])
                    nc.sync.dma_start(yT[dh * P : (dh + 1) * P, c0 : c0 + cw], dst)

        for p in reversed(pools):
            p.release()

    nc.compile()
    names = dict(
        w1=[t.name for t in w1s],
        w2=[t.name for t in w2s],
        xeT=xeT.name,
        y=yT.name,
    )
    return nc, names


def _get_program(a, b):
    if (a, b) not in _PROGRAMS:
        _PROGRAMS[(a, b)] = _build_program(a, b)
    return _PROGRAMS[(a, b)]


RUN_KWARGS = {}
LAST_RESULTS = None


def kernel(x, route_mask, route_weight, W1, b1, W2, b2):
    import ml_dtypes

    from concourse.bass_utils import run_bass_kernel_spmd

    global LAST_RESULTS

    bf = ml_dtypes.bfloat16

    x = np.asarray(x, dtype=np.float32)
    route_mask = np.asarray(route_mask, dtype=bool)
    route_weight = np.asarray(route_weight, dtype=np.float32)
    W1 = np.asarray(W1, dtype=np.float32)
    W2 = np.asarray(W2, dtype=np.float32)
    b1 = np.asarray(b1, dtype=np.float32)
    b2 = np.asarray(b2, dtype=np.float32)
    if np.any(b1):
        raise NotImplementedError("nonzero b1 not supported")

    w_et = np.where(route_mask.T, route_weight.T, -np.inf)  # (E, T)
    order = np.argsort(-w_et, axis=1, kind="stable")[:, :C]  # (E, C)
    vals = np.take_along_axis(w_et, order, axis=1)
    valid = np.isfinite(vals)
    gain = np.where(valid, vals, 0.0).astype(np.float32)

    n_e = np.minimum(valid.sum(axis=1), C).astype(int)
    a, b, a_segs, b_segs = _plan_segments(n_e)
    B = a + b

    nc, names = _get_program(a, b)

    # pre-pack per-expert weights once (an expert may appear on 2 cores)
    used = sorted({e for e, _, cnt in a_segs + b_segs if cnt > 0})
    w1p_, w2p_ = {}, {}
    for e in used:
        # (NF, P, KO, P) -> 2-fc packs (NF//2, P, 2, KO, P)
        w1e = W1[e].reshape(KO, P, NF, P).transpose(2, 1, 0, 3).astype(bf)
        w1p_[e] = np.ascontiguousarray(
            w1e.reshape(NF // 2, 2, P, KO, P).transpose(0, 2, 1, 3, 4)
        )
        w2p_[e] = np.ascontiguousarray(
            W2[e].reshape(NF, P, NDC, P).transpose(2, 1, 0, 3).astype(bf)
        )
    w1_pad = np.zeros((NF // 2, P, 2, KO, P), bf)
    w2_pad = np.zeros((NDC, P, NF, P), bf)

    in_maps = []
    for core in range(N_CORES):
        segs = [(a_segs[core], 0, a), (b_segs[core], a, b)]
        xeT_np = np.zeros((D, B), bf)
        im = {}
        for s, ((e, off, cnt), base, width) in enumerate(segs):
            if cnt > 0:
                idx = order[e, off : off + cnt]
                xe = x[idx] * gain[e, off : off + cnt][:, None]
                xeT_np[:, base : base + cnt] = xe.T.astype(bf)
                im[names["w1"][s]] = w1p_[e]
                im[names["w2"][s]] = w2p_[e]
            else:
                im[names["w1"][s]] = w1_pad
                im[names["w2"][s]] = w2_pad
        im[names["xeT"]] = np.ascontiguousarray(xeT_np)
        in_maps.append(im)

    res = run_bass_kernel_spmd(nc, in_maps, list(range(N_CORES)), **RUN_KWARGS)
    LAST_RESULTS = res

    y = np.zeros((T, D), np.float32)
    for core in range(N_CORES):
        yTc = res.results[core][names["y"]]
        for (e, off, cnt), base, width in (
            (a_segs[core], 0, a),
            (b_segs[core], a, b),
        ):
            if cnt == 0:
                continue
            ye = yTc[:, base : base + cnt].T.astype(np.float32)
            if np.any(b2):
                ye = ye + gain[e, off : off + cnt][:, None] * b2[e][None, :]
            y[order[e, off : off + cnt]] += ye
    return y
